# revision 32
# baseline (speedup 1.0000x reference)
"""Trainium2 Bass kernel for multi-head attention (nn_Attention).

Problem: x[8, 32, 32, 768] -> MHA(12 heads, d=64) -> out[8, 32, 32, 768].

Sharding: pure data parallel. Batch B=8 maps 1:1 onto the 8 NeuronCores;
weights are replicated. No collectives.

v2 design (vs the fp32-PE-transpose baseline at 397us):
  - All input transposes (x, qkv_w, proj_w) moved off the PE: DMA fp32 row
    tiles to SBUF, cast to bf16 on the idle GPSIMD engine, then SBUF->SBUF
    XBAR dma_start_transpose into [P, tile, ct, 128] layouts (each call's
    destination is per-partition contiguous, required by the XBAR path).
  - qkv bias loaded as one strided DMA into [128, 18] feature-major; the
    v-bias is folded into the proj bias (softmax rows sum to 1):
    pb' = pb + proj_w @ bv, computed with one tiny PE matmul chain.
  - Scores S^T = k^T.T @ q^T per head pair (2 heads packed in the 128-row
    PE array via tile_position), exp on ACT straight out of PSUM into bf16
    E tiles; PV accumulates [V|1].T @ E so the softmax denominator rides
    along as PSUM row 64.
  - Normalization never stalls the PE: reciprocal (DVE, direct from PSUM)
    is emitted right after the last PV matmul, an independent qkv
    projection unit runs on the PE while it completes, then the f32r
    ones-broadcast matmul + DVE multiply produce normalized OT.
  - Emission interleaves scores(p) / pv(p-1) per key tile with qkv/V
    projection filler units so the PE never sits on a single dependency
    and PSUM (2x 2-bank rotating pools + 2x 2-bank PV accumulators) always
    has a drained buffer ready.
"""

import os
import sys

for _p in ("/opt/trn_rl_repo",):
    if _p not in sys.path:
        sys.path.insert(0, _p)

import numpy as np

import concourse.bass as bass
from concourse import bacc
import concourse.mybir as mybir
from concourse.masks import make_identity
from concourse.tile import TileContext

F32 = mybir.dt.float32
F32R = mybir.dt.float32r
BF16 = mybir.dt.bfloat16

P = 128
C = 768            # model dim
CT = C // P        # 6 c-tiles
N = 1024           # tokens per batch element
NT = N // P        # 8 token tiles
HEADS = 12
D = 64
PAIRS = HEADS // 2  # 6
OT3 = 3 * C // P   # 18 qkv_w row tiles
SCALE = D ** -0.5  # 0.125


def build_nc() -> bass.Bass:
    nc = bacc.Bacc(None, target_bir_lowering=False)
    x_d = nc.declare_dram_parameter("x", [N, C], F32, isOutput=False)
    qkvw_d = nc.declare_dram_parameter("qkv_w", [3 * C, C], F32, isOutput=False)
    qkvb_d = nc.declare_dram_parameter("qkv_b", [3 * C], F32, isOutput=False)
    projw_d = nc.declare_dram_parameter("proj_w", [C, C], F32, isOutput=False)
    projb_d = nc.declare_dram_parameter("proj_b", [C], F32, isOutput=False)
    out_d = nc.declare_dram_parameter("out", [N, C], F32, isOutput=True)

    with TileContext(nc) as tc:
        with (
            tc.tile_pool(name="const", bufs=1) as cpool,
            tc.tile_pool(name="load", bufs=3) as lpool,
            tc.tile_pool(name="cast", bufs=4) as bfpool,
            tc.tile_pool(name="qk", bufs=2) as qkpool,
            tc.tile_pool(name="v", bufs=1) as vpool,
            tc.tile_pool(name="otp", bufs=1) as otpool,
            tc.tile_pool(name="xTp", bufs=1) as xtpool,
            tc.tile_pool(name="wTp", bufs=1) as wtpool,
            tc.tile_pool(name="pwp", bufs=1) as pwpool,
            tc.tile_pool(name="e", bufs=4) as epool,
            tc.tile_pool(name="rec", bufs=1) as rpool,
            tc.tile_pool(name="recr", bufs=2) as rrpool,
            tc.tile_pool(name="bc", bufs=2) as bcpool,
            tc.tile_pool(name="outs", bufs=2) as outpool,
            tc.tile_pool(name="oacc", bufs=1) as oaccpool,
            tc.tile_pool(name="psa", bufs=2, space="PSUM") as psa,
            tc.tile_pool(name="psv", bufs=2, space="PSUM") as psv,
        ):
            # ---------------- persistent tensors ----------------
            xT = xtpool.tile([P, NT, CT, P], BF16, tag="xT")     # x^T per nt
            WT = wtpool.tile([P, OT3, CT, P], BF16, tag="WT")    # qkv_w^T per ot
            PwT = pwpool.tile([P, CT, CT, P], BF16, tag="PwT")   # proj_w^T per c2t
            V = vpool.tile([P, NT, HEADS, D + 1], BF16, tag="V")  # token-major + ones
            OT = otpool.tile([P, CT, N], BF16, tag="OT")         # attn out, feat-major
            outacc = oaccpool.tile([P, NT, C], BF16, tag="oacc")  # proj partial
            QK = {}  # pair -> (q_tile, k_tile), feature-major [128, N]

            # ---------------- loads ----------------
            # identity for PE transposes (bf16, 1 cycle/row)
            ident = cpool.tile([P, P], BF16, tag="ident")
            make_identity(nc, ident)

            # DMA [128, C] fp32 -> cast bf16 (DVE in the prologue where it is
            # idle, slow-but-idle GPSIMD mid-run) -> 6 bf16 PE transposes into
            # one [128, C] bf16 PSUM tile -> single batched copy to the
            # [128, CT, 128] destination (DVE in prologue, ACT mid-run).
            # Mid-run tiles split the dma+cast (ld_start, a pair earlier)
            # from the PE transposes (ld_finish) so the PE never waits on a
            # 2.7us GPSIMD cast.
            pending = {}

            def ld_start(key, dram_rows, prologue):
                st = lpool.tile([P, C], F32, tag="ld")
                nc.sync.dma_start(st, dram_rows)
                bt = bfpool.tile([P, C], BF16, tag="cast")
                (nc.vector if prologue else nc.gpsimd).tensor_copy(bt, st)
                pending[key] = bt

            def ld_finish(key, dest3, prologue):
                bt = pending.pop(key)
                ps = psa.tile([P, C], BF16, tag="psa", name="ps_tr")
                for ct in range(CT):
                    nc.tensor.transpose(
                        ps[:, ct * P : (ct + 1) * P],
                        bt[:, ct * P : (ct + 1) * P],
                        ident,
                    )
                pr = ps.rearrange("p (a b) -> p a b", b=P)
                if prologue:
                    nc.vector.tensor_copy(dest3, pr)
                else:
                    nc.scalar.activation(
                        dest3, pr, mybir.ActivationFunctionType.Copy
                    )

            def ldx(nt):
                ld_start(("x", nt), x_d[nt * P : (nt + 1) * P, :], True)
                ld_finish(("x", nt), xT[:, nt], True)

            def ldw(ot, prologue=False):
                ld_start(("w", ot), qkvw_d[ot * P : (ot + 1) * P, :], prologue)
                if prologue:
                    ld_finish(("w", ot), WT[:, ot], True)

            def ldw_fin(ot):
                ld_finish(("w", ot), WT[:, ot], False)

            def ldpw(ct):
                ld_start(("pw", ct), projw_d[ct * P : (ct + 1) * P, :], False)

            def ldpw_fin(ct):
                ld_finish(("pw", ct), PwT[:, ct], False)

            # bias loads first: their tiny-descriptor DMA storm runs on a
            # parallel queue while the big row loads stream
            biasT = cpool.tile([P, OT3], F32, tag="biasT")
            nc.sync.dma_start(biasT, qkvb_d.rearrange("(t p) -> p t", p=P))
            pb_st = cpool.tile([1, C], F32, tag="pb_st")
            nc.sync.dma_start(pb_st, projb_d[None, :])

            for nt in range(NT):
                ldx(nt)
            ldw(0, True)
            ldw(6, True)
            ldw(12, True)

            # ---------------- constants ----------------
            ones_st = cpool.tile([1, P], F32, tag="ones_st")
            nc.gpsimd.memset(ones_st, 1.0)
            ones_bf = cpool.tile([1, P], BF16, tag="ones_bf")
            nc.vector.tensor_copy(ones_bf, ones_st)
            ones_r = cpool.tile([1, P], F32R, tag="ones_r")
            nc.vector.tensor_copy(ones_r, ones_st)
            nc.gpsimd.memset(V[:, :, :, D], 1.0)

            bvT = cpool.tile([P, CT], BF16, tag="bvT")
            nc.vector.tensor_copy(bvT, biasT[:, 2 * CT :])
            pbp = cpool.tile([1, C], BF16, tag="pbp")  # pb + Pw @ bv

            # ---------------- compute units ----------------
            def qk_proj(p, which):
                """Feature-major q (which=0) or k (which=1) projection for
                head pair p; allocates the pair's [128, N] tile."""
                ot = p + which * CT
                ps = psa.tile([P, N], F32, tag="psa", name=f"ps_qk{ot}")
                for ct in range(CT):
                    for ic in range(2):
                        nc.tensor.matmul(
                            ps[:, ic * 512 : (ic + 1) * 512],
                            WT[:, ot, ct, :],
                            xT[:, ic * 4 : ic * 4 + 4, ct, :],
                            start=(ct == 0),
                            stop=(ct == CT - 1),
                        )
                t = qkpool.tile(
                    [P, N], BF16, tag="kT" if which else "qT", name=f"qk{ot}"
                )
                QK.setdefault(p, [None, None])[which] = t
                nc.vector.tensor_scalar_add(t, ps, biasT[:, ot : ot + 1])

            def v_proj(vp, nt):
                """Token-major V for head pair vp, token tile nt (no bias --
                v bias is folded into the proj bias)."""
                ps = psa.tile([P, P], F32, tag="psa", name=f"ps_v{vp}_{nt}")
                for ct in range(CT):
                    nc.tensor.matmul(
                        ps,
                        xT[:, nt, ct, :],
                        WT[:, 2 * CT + vp, ct, :],
                        start=(ct == 0),
                        stop=(ct == CT - 1),
                    )
                nc.vector.tensor_copy(
                    V[:, nt, 2 * vp : 2 * vp + 2, 0:D],
                    ps.rearrange("p (h d) -> p h d", d=D),
                )

            def scores(pair, jt, half, E):
                lo = half * D
                qt, kt = QK[pair]
                ps = psa.tile([P, N], F32, tag="psa", name=f"ps_s{pair}_{jt}_{half}")
                for ic in range(2):
                    nc.tensor.matmul(
                        ps[:, ic * 512 : (ic + 1) * 512],
                        kt[lo : lo + D, jt * P : (jt + 1) * P],
                        qt[lo : lo + D, ic * 512 : (ic + 1) * 512],
                        start=True,
                        stop=True,
                        tile_position=(lo, 0),
                    )
                nc.scalar.activation(
                    E[:, jt, :], ps, mybir.ActivationFunctionType.Exp, scale=SCALE
                )

            def pv(h, jt, E, pspv):
                for ic in range(2):
                    nc.tensor.matmul(
                        pspv[0 : D + 1, ic * 512 : (ic + 1) * 512],
                        V[:, jt, h, :],
                        E[:, jt, ic * 512 : (ic + 1) * 512],
                        start=(jt == 0),
                        stop=(jt == NT - 1),
                    )

            def recip_den(h, pspv):
                den = rpool.tile([1, N], F32, tag="den", name=f"den{h}")
                nc.vector.tensor_copy(den, pspv[D : D + 1, :])
                rec_st = rpool.tile([1, N], F32, tag="rec_st", name=f"recs{h}")
                nc.vector.reciprocal_approx_fast(rec_st, den)
                rec = rrpool.tile([1, N], F32R, tag="rec", name=f"rec{h}")
                nc.vector.tensor_copy(rec, rec_st)
                return rec

            def normalize(h, pspv, rec):
                """bcast 1/den across 64 partitions (f32r PE matmul), then
                OT[h] = pspv[0:D] * bcast on DVE."""
                psbc = psa.tile([D, N], F32, tag="psa", name=f"ps_bc{h}")
                for ic in range(2):
                    nc.tensor.matmul(
                        psbc[:, ic * 512 : (ic + 1) * 512],
                        ones_r[:, 0:D],
                        rec[:, ic * 512 : (ic + 1) * 512],
                        start=True,
                        stop=True,
                    )
                bcsb = bcpool.tile([D, N], BF16, tag="bc", name=f"bc{h}")
                nc.vector.tensor_copy(bcsb, psbc)
                nc.vector.tensor_mul(
                    OT[(h % 2) * D : (h % 2) * D + D, h // 2, :], pspv[0:D, :], bcsb
                )

            def pb_fold():
                """pbp = proj_b + proj_w @ v_bias (one [1, C] PE chain)."""
                ps = psa.tile([1, C], F32, tag="psa", name="ps_pb")
                for hdt in range(CT):
                    for o0, ow in ((0, 512), (512, 256)):
                        nc.tensor.matmul(
                            ps[:, o0 : o0 + ow],
                            bvT[:, hdt : hdt + 1],
                            PwT[:, o0 // P : (o0 + ow) // P, hdt, :],
                            start=(hdt == 0),
                            stop=(hdt == CT - 1),
                        )
                nc.vector.tensor_add(pbp, ps, pb_st)

            def proj_a(it):
                """Output projection, head pairs 0..3, bias-seeded; partial
                sum parked in SBUF bf16."""
                ps = psa.tile([P, N], F32, tag="psa", name=f"ps_oa{it}")
                for o0, ow in ((0, 512), (512, 256)):
                    nc.tensor.matmul(
                        ps[:, o0 : o0 + ow], ones_bf, pbp[:, o0 : o0 + ow],
                        start=True, stop=False,
                    )
                for hdt in range(4):
                    for o0, ow in ((0, 512), (512, 256)):
                        nc.tensor.matmul(
                            ps[:, o0 : o0 + ow],
                            OT[:, hdt, it * P : (it + 1) * P],
                            PwT[:, o0 // P : (o0 + ow) // P, hdt, :],
                            start=False,
                            stop=(hdt == 3),
                        )
                nc.vector.tensor_copy(outacc[:, it], ps[:, :C])

            def proj_b(it):
                """Head pairs 4..5 + parked partial -> fp32 out, DMA."""
                outt = outpool.tile([P, C], F32, tag="out")
                ps = psa.tile([P, N], F32, tag="psa", name=f"ps_ob{it}")
                for hdt in (4, 5):
                    for o0, ow in ((0, 512), (512, 256)):
                        nc.tensor.matmul(
                            ps[:, o0 : o0 + ow],
                            OT[:, hdt, it * P : (it + 1) * P],
                            PwT[:, o0 // P : (o0 + ow) // P, hdt, :],
                            start=(hdt == 4),
                            stop=(hdt == 5),
                        )
                nc.vector.tensor_add(outt, ps[:, :C], outacc[:, it])
                nc.sync.dma_start(out_d[it * P : (it + 1) * P, :], outt)

            # ---------------- interleaved emission ----------------
            ldw(1, True)
            ldw(7, True)
            qk_proj(0, 0)
            qk_proj(0, 1)
            ldw(13, True)

            Es = {}

            def new_E(p):
                E0 = epool.tile([P, NT, N], BF16, tag="E", name=f"E0_{p}")
                E1 = epool.tile([P, NT, N], BF16, tag="E", name=f"E1_{p}")
                Es[p] = (E0, E1)

            # pair 0: scores only (no pv yet); qk(1) q/k as slot filler
            new_E(0)
            for jt in range(NT):
                scores(0, jt, 0, Es[0][0])
                if jt == 1:
                    qk_proj(1, 0)
                if jt == 4:
                    qk_proj(1, 1)
                if jt in (2, 3, 5, 6):
                    v_proj(0, jt - 2 if jt < 4 else jt - 3)
                scores(0, jt, 1, Es[0][1])
                if jt == 0:
                    ld_start(("w", 2), qkvw_d[2 * P : 3 * P, :], False)
                    ld_start(("w", 8), qkvw_d[8 * P : 9 * P, :], False)
                    ld_start(("w", 14), qkvw_d[14 * P : 15 * P, :], False)
            for nt in range(4, NT):
                v_proj(0, nt)

            # pairs 1..5: per key-tile slot [scores h0, pv A, scores h1,
            # pv B, v_proj] paced to the ACT exp drain rate; long qk units,
            # W transposes and normalization live in the post-loop window
            # where the scores PSUM ring is quiet.
            LD_SCHED = {  # pair -> (W tiles to start, pw tiles to start)
                1: ((3, 9, 15), ()),
                2: ((4, 10, 16), (0, 1, 2)),
                3: ((5, 11, 17), (3, 4, 5)),
            }
            FIN_SCHED = {  # pair -> (W tiles to finish, pw tiles to finish)
                0: ((2, 8, 14), ()),
                1: ((3, 9, 15), ()),
                2: ((4, 10, 16), (0, 1, 2)),
                3: ((5, 11, 17), (3, 4, 5)),
            }
            for p in range(1, PAIRS):
                new_E(p)
                hA, hB = 2 * (p - 1), 2 * (p - 1) + 1
                last = p == PAIRS - 1
                pspvA = psv.tile([D + 1, N], F32, tag="psv", name=f"pvA{p}")
                pspvB = psv.tile([D + 1, N], F32, tag="psv", name=f"pvB{p}")
                for jt in range(NT):
                    scores(p, jt, 0, Es[p][0])
                    pv(hA, jt, Es[p - 1][0], pspvA)
                    scores(p, jt, 1, Es[p][1])
                    pv(hB, jt, Es[p - 1][1], pspvB)
                    if last:
                        proj_a(jt)
                    else:
                        v_proj(p, jt)
                    if jt < 2 and p in LD_SCHED:
                        ws, pws = LD_SCHED[p]
                        items = ws if jt == 0 else ()
                        for ot in items:
                            ld_start(("w", ot), qkvw_d[ot * P : (ot + 1) * P, :], False)
                        if jt == 1:
                            for ct in pws:
                                ld_start(
                                    ("pw", ct), projw_d[ct * P : (ct + 1) * P, :], False
                                )
                recA = recip_den(hA, pspvA)
                recB = recip_den(hB, pspvB)
                if p - 1 in FIN_SCHED:
                    ws, pws = FIN_SCHED[p - 1]
                    for ot in ws:
                        ldw_fin(ot)
                    for ct in pws:
                        ldpw_fin(ct)
                if not last:
                    qk_proj(p + 1, 0)
                    qk_proj(p + 1, 1)
                if p == 4:
                    pb_fold()
                normalize(hA, pspvA, recA)
                normalize(hB, pspvB, recB)
                if last:
                    for nt in range(NT):
                        v_proj(p, nt)

            # final pair's PV (both heads interleaved per key tile),
            # normalize, then the remaining output projection chunk
            pspvA = psv.tile([D + 1, N], F32, tag="psv", name="pvA6")
            pspvB = psv.tile([D + 1, N], F32, tag="psv", name="pvB6")
            for jt in range(NT):
                pv(10, jt, Es[5][0], pspvA)
                pv(11, jt, Es[5][1], pspvB)
            recA = recip_den(10, pspvA)
            recB = recip_den(11, pspvB)
            normalize(10, pspvA, recA)
            normalize(11, pspvB, recB)
            for it in range(NT):
                proj_b(it)

    nc.compile()
    return nc


_NC_CACHE = None


def _get_nc():
    global _NC_CACHE
    if _NC_CACHE is None:
        _NC_CACHE = build_nc()
    return _NC_CACHE


def run(inputs, trace=False, tmpdir=None):
    """Run on 8 NeuronCores; returns (out[8,32,32,768], BassKernelResults)."""
    from concourse.bass_utils import run_bass_kernel_spmd

    x = np.asarray(inputs["x"], dtype=np.float32)
    B, H, W, Cc = x.shape
    xf = np.ascontiguousarray(x.reshape(B, H * W, Cc))
    qkv_w = np.ascontiguousarray(np.asarray(inputs["qkv_w"], dtype=np.float32))
    qkv_b = np.ascontiguousarray(np.asarray(inputs["qkv_b"], dtype=np.float32))
    proj_w = np.ascontiguousarray(np.asarray(inputs["proj_w"], dtype=np.float32))
    proj_b = np.ascontiguousarray(np.asarray(inputs["proj_b"], dtype=np.float32))

    nc = _get_nc()
    in_maps = [
        {
            "x": xf[b],
            "qkv_w": qkv_w,
            "qkv_b": qkv_b,
            "proj_w": proj_w,
            "proj_b": proj_b,
        }
        for b in range(B)
    ]
    res = run_bass_kernel_spmd(nc, in_maps, list(range(B)), trace=trace, tmpdir=tmpdir)
    out = np.stack([res.results[b]["out"] for b in range(B)])
    return out.reshape(B, H, W, Cc).astype(np.float32), res


def kernel(x, qkv_w, qkv_b, proj_w, proj_b):
    out, _ = run(
        {
            "x": x,
            "qkv_w": qkv_w,
            "qkv_b": qkv_b,
            "proj_w": proj_w,
            "proj_b": proj_b,
        }
    )
    return out
